# revision 75
# baseline (speedup 1.0000x reference)
"""MoE kernel for Trainium2 (8 NeuronCores) — 8-way feature-split.

Strategy (feature-parallel over DFF, skew-immune):
  - Host sorts the T=4096 tokens by dispatch_order into per-expert column
    blocks (padded to a multiple of 8, split at 768 so no block exceeds
    SBUF-friendly size).  EVERY core sees all token columns, but core c
    computes only its 4 of the 32 DFF f-chunks (512 of 4096 ff dims) for
    every expert:  h_f = gelu(x @ W1[:, f] + b1[f]) entirely on-core,
    partial y_c = sum_f h_f @ W2[f, :].  The host sums the 8 partial
    outputs and adds b2.  This is an exact decomposition; per-core compute
    is sum_e ceil8(count_e) columns instead of 8*max_e(count_e), and
    weight traffic is 12.6 MB/core (each core reads 1/8th of every
    expert's W1/W2; W1 travels as fp8 e3m4).
  - Device pipeline per expert block: phase 1 k-outer (PSUM holds the 4
    f-chunks while the 8 k-slabs stream in), then phase 2 dm-outer
    (yT partial = W2-slice @ h), partial y DMA'd out as float16.
  - Every DMA'd operand slab is its own Tile tile: the framework
    synchronizes DMA->compute per whole tile, so slab-granular tiles are
    what lets the PE chase the DMA stream k-slab by k-slab.

Self-contained: hardcodes all shapes from the problem spec.
"""

import os
import sys
from contextlib import ExitStack

import ml_dtypes
import numpy as np

for _p in ("/opt/trn_rl_repo",):
    if _p not in sys.path:
        sys.path.insert(0, _p)

import concourse.bass as bass  # noqa: E402
import concourse.tile as tile  # noqa: E402
from concourse import mybir  # noqa: E402
from concourse.bass_utils import run_bass_kernel_spmd  # noqa: E402

# ---------------------------------------------------------------------------
# Workaround for this walrus build: a Drain instruction with >1 sem wait
# fails codegen ("Too many sync wait commands").  Replace the Tile
# kernel-tail drain with single-wait SP nops followed by a bare drain.
# ---------------------------------------------------------------------------


def _patched_drain_and_barrier(self, tick_clock, wait_clock):
    from concourse.vector_clock import ScopedClock

    nc = self.nc
    probe = nc.sync.nop(nofuse=True)
    wait_clock.add_sem_waits(probe.ins, ScopedClock({None: tick_clock.global_clock}))
    si = probe.ins.sync_info
    waits = list(si.on_wait) if si and si.on_wait else []
    probe.ins.sync_info = mybir.SyncInfo(on_wait=waits[:1], on_update=[])
    for w in waits[1:]:
        n = nc.sync.nop(nofuse=True)
        n.ins.sync_info = mybir.SyncInfo(on_wait=[w], on_update=[])

    nc.sync.drain()
    nc.all_engine_barrier()
    assert self.sems is not None
    popped = nc._tile_sem_poison_stack.pop()
    assert popped is self._sem_poison
    nc.clear_and_free_semaphores(list(self.sems.allocated().values()))
    nc.all_engine_barrier()


tile.TileContext._drain_and_barrier = _patched_drain_and_barrier


def _thin_mm_sem_updates(nc, keep_names):
    """Every matmul carries a Tile-framework semaphore increment that costs
    ~26ns of serialized PE time (25% on top of a 264-col matmul's 110ns
    streaming).  Matmuls complete strictly in program order, so an increment
    is only needed where a consumer's threshold can land: keep the ones in
    `keep_names` (group stops + the last matmul of each k-slab batch), drop
    the rest, and rewrite every waiter's threshold t to the rank of the
    first kept increment at-or-after position t."""
    import bisect

    mms = []
    for f in nc.m.functions:
        for bb in f.blocks:
            for inst in bb.instructions:
                if isinstance(inst, mybir.InstMatmult):
                    mms.append(inst)

    # sems updated exclusively by matmuls (each +1, no register updates)
    sem_updaters = {}
    bad_sems = set()
    for f in nc.m.functions:
        for bb in f.blocks:
            for inst in bb.instructions:
                si = inst.sync_info
                if not si:
                    continue
                for u in si.on_update or []:
                    if u.sync_type != "semaphore":
                        continue
                    if (
                        isinstance(inst, mybir.InstMatmult)
                        and u.update_mode == "sem-inc"
                        and u.update_value == 1
                    ):
                        sem_updaters.setdefault(u.id, []).append(inst)
                    else:
                        bad_sems.add(u.id)
    for sem_id, updaters in sem_updaters.items():
        if sem_id in bad_sems or len(updaters) < 64:
            continue
        pos = {inst.name: i + 1 for i, inst in enumerate(updaters)}
        kept = sorted(pos[i.name] for i in updaters if i.name in keep_names)
        if not kept or kept[-1] != len(updaters):
            continue  # the final matmul must keep its increment
        # only ">= imm" waiters can be re-thresholded; skip the sem if any
        # other wait flavor references it
        ok = True
        for f in nc.m.functions:
            for bb in f.blocks:
                for inst in bb.instructions:
                    si = inst.sync_info
                    for w in (si.on_wait or []) if si else []:
                        if (
                            getattr(w, "id", None) == sem_id
                            and w.sync_type == "semaphore"
                            and w.wait_mode != "sem-ge-imm"
                        ):
                            ok = False
        if not ok:
            continue
        # rewrite waiter thresholds in place (SyncWait is mutable)
        for f in nc.m.functions:
            for bb in f.blocks:
                for inst in bb.instructions:
                    si = inst.sync_info
                    if not si or not si.on_wait:
                        continue
                    for w in si.on_wait:
                        if (
                            getattr(w, "id", None) == sem_id
                            and w.sync_type == "semaphore"
                            and w.wait_mode == "sem-ge-imm"
                        ):
                            t = w.wait_value
                            j = bisect.bisect_left(kept, t)
                            w.wait_value = j + 1 if j < len(kept) else len(kept)
        # drop increments from non-kept matmuls
        keptset = set(kept)
        for inst in updaters:
            if pos[inst.name] in keptset:
                continue
            si = inst.sync_info
            ups = [
                u
                for u in (si.on_update or [])
                if not (u.sync_type == "semaphore" and u.id == sem_id)
            ]
            inst.sync_info = mybir.SyncInfo(
                on_wait=list(si.on_wait or []), on_update=ups
            )


def _split_excess_sync_waits(nc, max_waits=1):
    """This walrus build only encodes one sem wait per instruction.  Hoist
    excess waits onto same-engine nops inserted immediately before."""
    for f in nc.m.functions:
        for bb in f.blocks:
            out = []
            for inst in bb.instructions:
                si = inst.sync_info
                if si and si.on_wait and len(si.on_wait) > max_waits:
                    waits = list(si.on_wait)
                    for i in range(max_waits, len(waits), max_waits):
                        n = mybir.InstNoOp(
                            name=f"{inst.name}-waitsplit-{i}", ins=[], outs=[]
                        )
                        n.engine = inst.engine
                        n.sync_info = mybir.SyncInfo(
                            on_wait=waits[i : i + max_waits], on_update=[]
                        )
                        out.append(n)
                    inst.sync_info = mybir.SyncInfo(
                        on_wait=waits[:max_waits], on_update=list(si.on_update or [])
                    )
                out.append(inst)
            bb.instructions[:] = out


# ---------------------------------------------------------------------------

NUM_EXPERTS = 8
D = 1024
DFF = 4096
N_CORES = 8
KD = D // 128  # 8 contraction chunks for matmul 1
DM = D // 128  # 8 output chunks for matmul 2
FPC = (DFF // 128) // N_CORES  # 4 f-chunks per core
WCOL = KD * FPC * 128  # 4096 packed weight columns per block (w1 and w2)
SCAP = 768  # max token-block width; bigger experts split into several blocks

F32 = mybir.dt.float32
F16 = mybir.dt.float16
F8 = mybir.dt.float8e3  # e3m4: 4 mantissa bits, max finite 15.5

# w1 is stored in fp8 e3m4 scaled by W1_SCALE (power of 2, descaled exactly
# via the activation's pre-gelu scale).  Halves the w1 DMA traffic and
# LDWEIGHTS cost; measured end-to-end rel err 1.4e-2 vs the 2e-2 gate.
W1_SCALE = 128.0

# Dummy matmuls issued at kernel start to unthrottle the PE HAM clock gate
# (~213ns each at the cold 1.2 GHz clock).  The PE engine preamble ends at
# ~7us but the DGE rings deliver no input data before ~10.5us, so 16
# warmups exactly fill that dead window and the first real matmul runs at
# the full 2.4 GHz clock.
WARM_MMS = 13

# Short dummy-matmul bursts at the head of early p1 blocks: if a ramp
# feed stall does occur, an uninterrupted >3.4us PE-idle window would
# re-throttle the HAM clock to 1.2 GHz for >=3.4us more; a burst splits
# such windows.  With the depth-4 ramp the stalls should not occur; keep a
# small burst as insurance (it costs nothing while the ramp is DMA-bound).
KEEPALIVE_MMS = 0

# p1 phases run before the first p2: a deep ramp keeps w2 out of the
# DMA-critical first ~30us (the rings deliver ~350 GB/s aggregate but only
# from ~10us in; p1 phases need just x+w1 at ~230 GB/s).
PIPE_DEPTH = 4

LAST_EXEC_NS = None
LAST_RESULT = None

_NC_CACHE = {}


def _chunks(S):
    """Split S columns into <=512-wide chunks (PSUM bank limit), sizes
    multiple of 8, all >=256 when S allows (hides LDWEIGHTS)."""
    n = max(1, -(-S // 512))
    base = -(-(-(-S // n)) // 8) * 8
    out = []
    c0 = 0
    while c0 < S:
        cn = min(base, S - c0)
        out.append((c0, cn))
        c0 += cn
    return out


def _build_nc(sizes):
    nb = len(sizes)
    C = sum(sizes)
    nc = bass.Bass()
    xk = nc.declare_dram_parameter("xk", [128, KD * C], F16, isOutput=False)
    w1 = nc.declare_dram_parameter("w1", [128, nb * WCOL], F8, isOutput=False)
    w2 = nc.declare_dram_parameter("w2", [128, nb * WCOL], F16, isOutput=False)
    b1 = nc.declare_dram_parameter("b1", [128, nb * FPC], F32, isOutput=False)
    yk = nc.declare_dram_parameter("yk", [128, DM * C], F16, isOutput=True)

    gelu = mybir.ActivationFunctionType.Gelu_apprx_tanh
    xoff = [0]
    for S in sizes:
        xoff.append(xoff[-1] + KD * S)
    yoff = [0]
    for S in sizes:
        yoff.append(yoff[-1] + DM * S)

    with ExitStack() as ctx:
        tc = ctx.enter_context(tile.TileContext(nc))
        cpool = ctx.enter_context(tc.tile_pool(name="const", bufs=1))
        w1pool = ctx.enter_context(tc.tile_pool(name="w1", bufs=PIPE_DEPTH + 2))
        w2pool = ctx.enter_context(tc.tile_pool(name="w2", bufs=3))
        xpool = ctx.enter_context(tc.tile_pool(name="x", bufs=3))
        ypool = ctx.enter_context(tc.tile_pool(name="y", bufs=3))
        hpool = ctx.enter_context(tc.tile_pool(name="h", bufs=PIPE_DEPTH * FPC))
        pspool = ctx.enter_context(tc.tile_pool(name="ps", bufs=4, space="PSUM"))

        xts, w1ts, w2ts = {}, {}, {}

        def prefetch_xw1(e):
            # Every dma_start costs ~610ns of issuing-engine time (the DGE
            # descriptor write), so trigger count is a hard budget.  Tile
            # synchronizes DMA->compute per whole tile, so only the
            # ramp-critical blocks get fine-grained tiles; later blocks are
            # prefetched phases ahead and whole-tile waits are free there.
            # Ring loads during the ramp are balanced within ~0.1 MB: sync
            # carries w1(0)+half of x0/x1+w1(2..); scalar carries the other
            # x halves, b1 and w1(1).
            if e >= nb or e in xts:
                return
            S = sizes[e]
            w1t = w1pool.tile([128, WCOL], F8, name="w1t", tag="w1", bufs=PIPE_DEPTH + 2)
            # All w1 on the sync HWDGE ring.  (Tried: gpsimd software-DGE as
            # a third channel -- it only sustains ~35 GB/s per transfer and
            # made w1(2+) miss its deadline, +6us.)
            nc.sync.dma_start(w1t[:], w1[:, e * WCOL : (e + 1) * WCOL])
            if e <= 1:
                # both ramp blocks' 8 k-pair tiles are live before p1(0) runs;
                # alternating rings drains each block at the aggregate rate.
                # (Tried per-slab singles for block 0: the extra triggers
                # serialize in the DGE pipeline, +8us.)
                pairs = [
                    xpool.tile([128, 2 * S], F16, name="xt", tag="xf", bufs=KD)
                    for _ in range(KD // 2)
                ]
                for j in range(KD // 2):
                    q = nc.scalar if j % 2 == 0 else nc.sync
                    q.dma_start(
                        pairs[j][:],
                        xk[:, xoff[e] + 2 * j * S : xoff[e] + (2 * j + 2) * S],
                    )
                xs = [pairs[k // 2][:, (k % 2) * S : (k % 2 + 1) * S] for k in range(KD)]
            elif e <= 3:
                halves = [
                    xpool.tile([128, 4 * S], F16, name="xt", tag="xh", bufs=4)
                    for _ in range(2)
                ]
                for j in range(2):
                    nc.scalar.dma_start(
                        halves[j][:],
                        xk[:, xoff[e] + 4 * j * S : xoff[e] + (4 * j + 4) * S],
                    )
                xs = [halves[k // 4][:, (k % 4) * S : (k % 4 + 1) * S] for k in range(KD)]
            else:
                xt = xpool.tile([128, KD * S], F16, name="xt", tag="x")
                nc.scalar.dma_start(xt[:], xk[:, xoff[e] : xoff[e + 1]])
                xs = [xt[:, k * S : (k + 1) * S] for k in range(KD)]
            xts[e] = xs
            w1ts[e] = w1t

        def prefetch_w2(e):
            # w2 is packed dm-major on the host; p2 consumes it dm-by-dm.
            if e >= nb or e in w2ts:
                return
            w2t = w2pool.tile([128, WCOL], F16, name="w2t", tag="w2")
            nc.sync.dma_start(w2t[:], w2[:, e * WCOL : (e + 1) * WCOL])
            w2ts[e] = w2t

        warm_in = None
        keep_names = set()

        def dummy_mms(n):
            nonlocal warm_in
            if warm_in is None:
                warm_in = cpool.tile([128, 256], F16, name="warm_in")
                nc.vector.memset(warm_in[:], 0.0)
            wps = [
                pspool.tile([128, 256], F32, name="warm_ps", tag=("p1", "p2")[i])
                for i in range(2)
            ]
            for i in range(n):
                cc = nc.tensor.matmul(
                    wps[i % 2][:, :],
                    warm_in[:, :128],
                    warm_in[:, :],
                    start=True,
                    stop=True,
                )
                # dummies must keep their sem increments: following real
                # matmuls carry PSUM-ring WAR waits whose thresholds land on
                # them, and rounding those up to a later real matmul on the
                # same queue would deadlock the PE against itself.
                keep_names.add(cc.ins.name)

        if WARM_MMS:
            dummy_mms(WARM_MMS)

        # Triggers are queued per ring in phase-need order -- the DGE rings
        # drain strictly in trigger order, so an out-of-order entry causes
        # head-of-line blocking of a nearer-deadline transfer.
        prefetch_xw1(0)
        b1_sb = cpool.tile([128, nb * FPC], F32, name="b1_sb")
        # b1 rides the gpsimd software-DGE queue: small enough (~0.13 MB)
        # that the slow SWDGE still beats its ~15us deadline (first ACT),
        # and it frees the scalar ring's head for x1 in the scarcest window.
        nc.gpsimd.dma_start(b1_sb[:], b1[:, :])
        prefetch_xw1(1)
        for _j in range(2, min(PIPE_DEPTH, nb)):
            prefetch_xw1(_j)
        prefetch_w2(0)
        prefetch_xw1(min(PIPE_DEPTH, nb))
        # Dummy gelu on a preamble-memset const tile: pulls the scalar
        # engine's lazy ACT_TABLE_LOAD (~1.5us) off the critical path
        # (emitted after the ramp triggers so it does not delay them; the
        # first REAL activation gates p1(1)'s PSUM recycling).
        warm_out = cpool.tile([128, 1], F16, name="warm_out")
        nc.scalar.activation(
            warm_out[:],
            nc.const_aps.tensor(0.0, (128, 1), F32),
            gelu,
            bias=0.0,
            scale=1.0,
        )

        hs_map = {}

        def do_p1(e):
            S = sizes[e]
            xs, w1t = xts.pop(e), w1ts.pop(e)
            chunks = _chunks(S)
            if KEEPALIVE_MMS and e == 1:
                dummy_mms(KEEPALIVE_MMS)

            # ---- phase 1: h_f = gelu(x @ W1[:,f] + b1[f]), k-outer ----
            # Chunks processed in groups of 2 using both PSUM tag rings (8
            # banks): doubles the PE work per arriving k-slab.
            hs = [hpool.tile([128, S], F16, name="h", tag="h") for _ in range(FPC)]
            for g0 in range(0, len(chunks), 2):
                grp = chunks[g0 : g0 + 2]
                pss = [
                    [
                        pspool.tile(
                            [128, cn], F32, name="ps1", tag=("p1", "p2")[gi]
                        )
                        for f in range(FPC)
                    ]
                    for gi, (c0, cn) in enumerate(grp)
                ]
                for k in range(KD):
                    for f in range(FPC):
                        for gi, (c0, cn) in enumerate(grp):
                            cc = nc.tensor.matmul(
                                pss[gi][f][:, :],
                                w1t[:, k * 512 + f * 128 : k * 512 + (f + 1) * 128],
                                xs[k][:, c0 : c0 + cn],
                                start=(k == 0),
                                stop=(k == KD - 1),
                            )
                            # keep sem increments only on group stops (exact
                            # ACT waits).  Mid-block increments are never
                            # needed: the fine x tiles (tags xf/xh) have as
                            # many buffers as allocations and never wrap, so
                            # every ring-recycle wait lands on a block-end
                            # stop anyway.
                            if k == KD - 1:
                                keep_names.add(cc.ins.name)
                for gi, (c0, cn) in enumerate(grp):
                    for f in range(FPC):
                        nc.scalar.activation(
                            hs[f][:, c0 : c0 + cn],
                            pss[gi][f][:, :],
                            gelu,
                            bias=b1_sb[:, e * FPC + f : e * FPC + f + 1],
                            scale=1.0 / W1_SCALE,
                        )
            hs_map[e] = hs

        def do_p2(e):
            S = sizes[e]
            w2t = w2ts.pop(e)
            hs = hs_map.pop(e)
            chunks = _chunks(S)

            # ---- phase 2: y_partial = sum_f h_f @ W2[f,:], dm-outer ----
            yt = ypool.tile([128, DM * S], F16, name="yt", tag="y")
            last = e == nb - 1
            gidx = 0
            for dm in range(DM):
                for ci, (c0, cn) in enumerate(chunks):
                    ps2 = pspool.tile([128, cn], F32, name="ps2", tag="p2")
                    for f in range(FPC):
                        cc = nc.tensor.matmul(
                            ps2[:, :],
                            w2t[:, dm * 512 + f * 128 : dm * 512 + (f + 1) * 128],
                            hs[f][:, c0 : c0 + cn],
                            start=(f == 0),
                            stop=(f == FPC - 1),
                        )
                        # Every group stop keeps its increment: rounding a
                        # copy's wait to the NEXT group's stop measured +24us
                        # on HW (the copy then gates the ps2 ring later than
                        # the model's slack estimate allows).
                        if f == FPC - 1:
                            keep_names.add(cc.ins.name)
                    gidx += 1
                    # The last block's copies alternate DVE and the (by now
                    # idle) ACT engine so the final two drain in parallel,
                    # and its dm>=6 y slices ship per chunk via sync-only
                    # triggers (a scalar trigger would delay the scalar
                    # copies behind it): the kernel-tail transfer is one
                    # ~120KB single.
                    if last and (dm + ci) % 2 == 1:
                        nc.scalar.activation(
                            yt[:, dm * S + c0 : dm * S + c0 + cn],
                            ps2[:, :],
                            mybir.ActivationFunctionType.Copy,
                            bias=0.0,
                            scale=1.0,
                        )
                    else:
                        nc.vector.tensor_scalar_add(
                            yt[:, dm * S + c0 : dm * S + c0 + cn], ps2[:, :], 0.0
                        )
                    if last and dm >= 6:
                        # sync-only triggers so the scalar engine stays free
                        # for the tail copies
                        nc.sync.dma_start(
                            yk[:, yoff[e] + dm * S + c0 : yoff[e] + dm * S + c0 + cn],
                            yt[:, dm * S + c0 : dm * S + c0 + cn],
                        )
                if last:
                    if dm < 6 and dm % 2 == 1:
                        yq = nc.scalar if e % 2 == 0 else nc.sync
                        yq.dma_start(
                            yk[:, yoff[e] + (dm - 1) * S : yoff[e] + (dm + 1) * S],
                            yt[:, (dm - 1) * S : (dm + 1) * S],
                        )
                elif e == nb - 2 and dm % 2 == 1:
                    # second-to-last block ships pairs during its p2 so its
                    # 1MB does not drain against the kernel-tail barrier
                    yq = nc.scalar if e % 2 == 0 else nc.sync
                    yq.dma_start(
                        yk[:, yoff[e] + (dm - 1) * S : yoff[e] + (dm + 1) * S],
                        yt[:, (dm - 1) * S : (dm + 1) * S],
                    )
            if e < nb - 2:
                yts[e] = yt

        yts = {}

        def flush_y(e):
            if e not in yts:
                return
            yt = yts.pop(e)
            yq = nc.scalar if e % 2 == 0 else nc.sync
            yq.dma_start(yk[:, yoff[e] : yoff[e + 1]], yt[:, :])

        # Software pipeline, ramp depth PIPE_DEPTH: p1(0..D-1) run before
        # p2(0), so the DMA-critical first ~30us only has to deliver x+w1
        # (~230 GB/s at PE pace) while w2 streams in behind; then p2/p1
        # phases alternate.  The tail is D consecutive p2 phases whose y
        # transfers alternate rings.
        depth = min(PIPE_DEPTH, nb)
        for j in range(depth):
            do_p1(j)
        for e in range(nb):
            # inputs first: w2/x have ~10us deadlines, the deferred y flush
            # has ~25us of yt-ring slack, so it must queue behind them
            prefetch_w2(e + 1)
            prefetch_xw1(e + depth + 1)
            flush_y(e - 1)
            do_p2(e)
            if e + depth < nb:
                do_p1(e + depth)
        flush_y(nb - 2)

    _thin_mm_sem_updates(nc, keep_names)
    _split_excess_sync_waits(nc)
    return nc


def _enable_trace_hooks():
    """Register the NTFF profile hook (missing antenv.axon_hooks shim)."""
    import types

    if "antenv.axon_hooks" not in sys.modules:
        mod = types.ModuleType("antenv.axon_hooks")
        mod._hook = None

        def set_axon_ntff_profile_hook(h):
            mod._hook = h

        def get_axon_ntff_profile_hook():
            return mod._hook

        mod.set_axon_ntff_profile_hook = set_axon_ntff_profile_hook
        mod.get_axon_ntff_profile_hook = get_axon_ntff_profile_hook
        sys.modules["antenv.axon_hooks"] = mod
        import antenv

        antenv.axon_hooks = mod
    import antenv.axon_hooks as ah

    if ah.get_axon_ntff_profile_hook() is None:
        from trn_agent_boot.trn_boot import _ntff_profile_via_ctypes

        ah.set_axon_ntff_profile_hook(
            _ntff_profile_via_ctypes("/opt/axon/libaxon_pjrt.so")
        )
    import concourse.bass_utils as bu

    bu.upload_artifacts = lambda tmpdir: "local://skipped"


def kernel(inputs, w1, b1, w2, b2, dispatch_order):
    global LAST_EXEC_NS, LAST_RESULT

    inputs = np.asarray(inputs, dtype=np.float32)
    w1 = np.asarray(w1, dtype=np.float32)
    b1 = np.asarray(b1, dtype=np.float32)
    w2 = np.asarray(w2, dtype=np.float32)
    b2 = np.asarray(b2, dtype=np.float32)
    disp = np.asarray(dispatch_order).astype(np.int64)

    B, Sq, _ = inputs.shape
    T = B * Sq
    x = inputs.reshape(T, D)

    order = np.argsort(disp, kind="stable")
    counts = np.bincount(disp, minlength=NUM_EXPERTS)
    starts = np.zeros(NUM_EXPERTS + 1, dtype=np.int64)
    np.cumsum(counts, out=starts[1:])

    # blocks: (expert, token-index-array) pairs; experts with more than SCAP
    # tokens split into several blocks sharing the expert's weights.
    # Processed big->small so the kernel tail handles the smallest block.
    entries = []
    for e in range(NUM_EXPERTS):
        toks = order[starts[e] : starts[e + 1]]
        for c0 in range(0, len(toks), SCAP):
            entries.append((e, toks[c0 : c0 + SCAP]))
    # Block order tuned to the DMA supply curve: the rings deliver least in
    # the first ~15us, so the two ramp blocks are the 2nd/3rd-smallest; the
    # big blocks sit in the supply-rich middle; the smallest block runs last
    # so the kernel-tail p2 and final y transfer are minimal.
    entries.sort(key=lambda et: (len(et[1]), et[0]))
    if len(entries) > 3:
        entries = entries[1:3] + entries[:0:-1][: len(entries) - 3] + entries[:1]
    blocks = [e for e, _ in entries]
    tok_lists = [t for _, t in entries]
    sizes = tuple(int(-(-len(t) // 8) * 8) for t in tok_lists)
    offs = np.zeros(len(sizes) + 1, dtype=np.int64)
    np.cumsum(sizes, out=offs[1:])
    C = int(offs[-1])

    key = sizes
    if key not in _NC_CACHE:
        _NC_CACHE[key] = _build_nc(sizes)
    nc = _NC_CACHE[key]

    # ---- pack x: per block, [128, KD*S] k-inner slabs, concatenated ----
    xk_arr = np.zeros((128, KD * C), dtype=np.float16)
    for bi, toks in enumerate(tok_lists):
        S = sizes[bi]
        xb = np.zeros((128, KD, S), dtype=np.float16)
        # x[toks] is [n, 1024]; feature dim k*128+p -> [k, p, n] -> [p, k, n]
        xb[:, :, : len(toks)] = (
            x[toks].T.reshape(KD, 128, len(toks)).transpose(1, 0, 2)
        )
        xk_arr[:, KD * offs[bi] : KD * offs[bi + 1]] = xb.reshape(128, KD * S)

    # ---- per-core weight packs: core c owns f-chunks [c*FPC, (c+1)*FPC) ----
    nb = len(blocks)
    w1_blocks = w1[blocks]  # [nb, 1024, 4096]
    w2_blocks = w2[blocks]  # [nb, 4096, 1024]
    b1_blocks = b1[blocks]  # [nb, 4096]
    in_maps = []
    for c in range(N_CORES):
        ff = slice(c * FPC * 128, (c + 1) * FPC * 128)
        # w1p[p, e*WCOL + k*512 + fl*128 + j] = w1[e][k*128+p, ff.start+fl*128+j]
        # Scaled by W1_SCALE into the e3m4 normal range (max |w1|*128 ~ 13.9
        # vs max finite 15.5); the kernel descales in the gelu activation.
        w1p = np.ascontiguousarray(
            np.clip(
                w1_blocks[:, :, ff]
                .reshape(nb, KD, 128, FPC, 128)
                .transpose(2, 0, 1, 3, 4)
                .reshape(128, nb * WCOL)
                * W1_SCALE,
                -15.5,
                15.5,
            )
        ).astype(ml_dtypes.float8_e3m4)
        # w2p[p, e*WCOL + dm*512 + fl*128 + j] = w2[e][ff.start+fl*128+p, dm*128+j]
        # (dm-major so p2 consumes w2 slabs in DMA arrival order)
        w2p = np.ascontiguousarray(
            w2_blocks[:, ff, :]
            .reshape(nb, FPC, 128, DM, 128)
            .transpose(2, 0, 3, 1, 4)
            .reshape(128, nb * WCOL)
        ).astype(np.float16)
        # b1p[p, e*FPC + fl] = b1[e][ff.start + fl*128 + p]
        b1p = np.ascontiguousarray(
            b1_blocks[:, ff].reshape(nb, FPC, 128).transpose(2, 0, 1).reshape(
                128, nb * FPC
            )
        ).astype(np.float32)
        in_maps.append({"xk": xk_arr, "w1": w1p, "w2": w2p, "b1": b1p})

    trace = os.environ.get("MOE_TRACE") == "1"
    kwargs = {}
    if trace:
        _enable_trace_hooks()
        kwargs["trace"] = True
        tmpdir = os.environ.get("MOE_TRACE_DIR")
        if tmpdir:
            os.makedirs(tmpdir, exist_ok=True)
            kwargs["tmpdir"] = tmpdir

    res = run_bass_kernel_spmd(nc, in_maps, list(range(N_CORES)), **kwargs)
    LAST_RESULT = res
    LAST_EXEC_NS = res.exec_time_ns

    # ---- gather: sum the 8 partial outputs, add b2, unsort ----
    ysum = np.zeros((128, DM * C), dtype=np.float32)
    for c in range(N_CORES):
        ysum += res.results[c]["yk"].astype(np.float32)

    out = np.empty((T, D), dtype=np.float32)
    for bi, e in enumerate(blocks):
        toks = tok_lists[bi]
        S = sizes[bi]
        yb = (
            ysum[:, DM * offs[bi] : DM * offs[bi + 1]]
            .reshape(128, DM, S)
            .transpose(1, 0, 2)
            .reshape(D, S)
        )
        out[toks] = yb[:, : len(toks)].T + b2[e][None, :]
    return out.reshape(B, Sq, D)
